# revision 10
# baseline (speedup 1.0000x reference)
"""Multi-head attention TRN2 Bass kernel.

Problem: B=2, S=2048, D_MODEL=1024, N_HEAD=16, D_HEAD=64.
  qh = split_heads(q @ Wq.T + bq) / 8;  kh, vh likewise (no scale)
  scores = (qh @ kh.T) * mask;  attn = softmax(scores);  ctx = attn @ vh
  out = ctx.reshape(B, S, 1024)   (raw [b,h,s,d] view, no head transpose-back)

Sharding (8 cores, tensor-parallel over heads + data-parallel over batch):
  core c -> batch b = c//4, heads [4*(c%4), 4*(c%4)+4)  (256 feature cols).

Per-core dataflow (all matmuls bf16 inputs, f32 PSUM accumulation):
  - host feeds x.T (bf16), W.T slices (bf16), mask.T (f32): no on-chip
    transposes needed anywhere.
  - Q.T in [dh, s] layout packed per head-pair; K.T per head zero-padded to
    128 contraction rows (K=64 matmuls stream the moving operand at half
    rate, so padding the stationary with zeros doubles throughput).
  - V in natural [s, f] layout with a ones column per head so the attn@V
    matmul also emits the softmax denominator.
  - scores.T tiles [sk=128, sq=512] via PE; mask multiply on DVE from PSUM
    into f32 staging; Exp on ACT into bf16 probs; ctx.T = (V|1).T @ probs.T
    accumulates over sk tiles with V|1 stationary (no per-tile weight
    reloads); PSUM->SBUF on ACT; normalization (divide by ones-column row)
    happens on the host after gather.
"""

import numpy as np
import ml_dtypes
from contextlib import ExitStack

import concourse.bass as bass
import concourse.mybir as mybir
import concourse.tile as tile
from concourse import bacc
from concourse.bass_utils import run_bass_kernel_spmd

BF16 = ml_dtypes.bfloat16

B = 2
D_MODEL = 1024
N_HEAD = 16
D_HEAD = 64
N_CORES = 8
HEADS_PER_CORE = 4
F = HEADS_PER_CORE * D_HEAD  # 256 feature cols per core
KC = D_MODEL // 128  # 8 contraction chunks

_NC_CACHE = {}


def build_nc(S=2048):
    f32 = mybir.dt.float32
    bf16 = mybir.dt.bfloat16
    Ident = mybir.ActivationFunctionType.Identity
    Exp = mybir.ActivationFunctionType.Exp

    SQC = min(512, S)        # sq chunk (one scores matmul N)
    NSC = S // SQC           # number of sq chunks
    NT = S // 128            # number of sk tiles
    EG = 4                   # sk tiles per Exp batch
    NEG = (NT + EG - 1) // EG
    D1 = D_HEAD + 1          # head cols incl. denominator ones column

    nc = bacc.Bacc("TRN2", target_bir_lowering=False, debug=False)

    xqT = nc.dram_tensor("xqT", [NSC, 128, KC, SQC], bf16, kind="ExternalInput").ap()
    xkT = nc.dram_tensor("xkT", [NSC, 128, KC, SQC], bf16, kind="ExternalInput").ap()
    xvT = nc.dram_tensor("xvT", [NSC, 128, KC, SQC], bf16, kind="ExternalInput").ap()
    wqT = nc.dram_tensor("wqT", [128, KC, F], bf16, kind="ExternalInput").ap()
    wkT = nc.dram_tensor("wkT", [128, KC, F], bf16, kind="ExternalInput").ap()
    wvT = nc.dram_tensor("wvT", [128, KC, F], bf16, kind="ExternalInput").ap()
    bqs = nc.dram_tensor("bqs", [128, 2], f32, kind="ExternalInput").ap()
    bks = nc.dram_tensor("bks", [128, 2], f32, kind="ExternalInput").ap()
    bvr = nc.dram_tensor("bvr", [1, F], f32, kind="ExternalInput").ap()
    maskT = nc.dram_tensor("maskT", [NSC, 128, NT, SQC], f32, kind="ExternalInput").ap()
    # per-head transposed context incl. denominator row (normalized on host)
    out = nc.dram_tensor(
        "out", [HEADS_PER_CORE, D1, S], f32, kind="ExternalOutput"
    ).ap()


    with tile.TileContext(nc) as tc, ExitStack() as ctx:
        const = ctx.enter_context(tc.tile_pool(name="const", bufs=1))
        persist = ctx.enter_context(tc.tile_pool(name="persist", bufs=1))
        xpool = ctx.enter_context(tc.tile_pool(name="xpool", bufs=2))
        mpool = ctx.enter_context(tc.tile_pool(name="mpool", bufs=2))
        fpool = ctx.enter_context(tc.tile_pool(name="fpool", bufs=2))
        ppool = ctx.enter_context(tc.tile_pool(name="ppool", bufs=2))
        copool = ctx.enter_context(tc.tile_pool(name="copool", bufs=3))
        # PSUM: 3 (proj) + 3 (scores) + 2 (ctx) = 8 banks
        psP = ctx.enter_context(tc.tile_pool(name="psP", bufs=3, space="PSUM"))
        psC = ctx.enter_context(tc.tile_pool(name="psC", bufs=3, space="PSUM"))
        psD = ctx.enter_context(tc.tile_pool(name="psD", bufs=2, space="PSUM"))

        # --- constants -----------------------------------------------------
        wq_sb = const.tile([128, KC, F], bf16)
        nc.sync.dma_start(wq_sb[:], wqT)
        wk_sb = const.tile([128, KC, F], bf16)
        nc.sync.dma_start(wk_sb[:], wkT)
        wv_sb = const.tile([128, KC, F], bf16)
        nc.sync.dma_start(wv_sb[:], wvT)
        bq_sb = const.tile([128, 2], f32)
        nc.sync.dma_start(bq_sb[:], bqs)
        bk_sb = const.tile([128, 2], f32)
        nc.sync.dma_start(bk_sb[:], bks)
        bv_sb = const.tile([1, F], f32)
        nc.sync.dma_start(bv_sb[:], bvr)

        ones1 = const.tile([1, 128], f32)
        nc.vector.memset(ones1[:], 1.0)
        bvp = psP.tile([128, F], f32, tag="pp")
        nc.tensor.matmul(bvp[:], ones1[:], bv_sb[:], start=True, stop=True)
        bvRep = const.tile([128, F], f32)
        nc.vector.tensor_copy(bvRep[:], bvp[:])

        QT_sb = persist.tile([128, 2, S], bf16)
        # K.T zero-padded per head: head h lives in rows (h%2)*64..+64 of
        # KTz[:, h, :], the other 64 rows stay zero -> scores run at K=128.
        KTz = persist.tile([128, HEADS_PER_CORE, S], bf16)
        for h in range(HEADS_PER_CORE):
            zr = (1 - h % 2) * 64
            nc.gpsimd.memset(KTz[zr : zr + 64, h, :], 0.0)
        V_sb = persist.tile([128, NT, HEADS_PER_CORE * D1], bf16)
        V4 = V_sb.rearrange("p t (h j) -> p t h j", h=HEADS_PER_CORE)
        nc.gpsimd.memset(V4[:, :, :, D_HEAD : D_HEAD + 1], 1.0)

        # --- K projection (+ head-split into KTz halves, bias on ACT) ------
        for sc in range(NSC):
            xk_t = xpool.tile([128, KC, SQC], bf16, tag="xt")
            nc.sync.dma_start(xk_t[:], xkT[sc])
            for ft in range(2):
                ps = psP.tile([128, SQC], f32, tag="pp")
                for kc in range(KC):
                    nc.tensor.matmul(
                        ps[:],
                        wk_sb[:, kc, ft * 128 : (ft + 1) * 128],
                        xk_t[:, kc, :],
                        start=(kc == 0),
                        stop=(kc == KC - 1),
                    )
                sl = slice(sc * SQC, (sc + 1) * SQC)
                nc.scalar.activation(
                    KTz[0:64, 2 * ft, sl], ps[0:64, :], Ident, bias=bk_sb[0:64, ft : ft + 1]
                )
                nc.scalar.activation(
                    KTz[64:128, 2 * ft + 1, sl],
                    ps[64:128, :],
                    Ident,
                    bias=bk_sb[64:128, ft : ft + 1],
                )

        # --- Q projection chunk maker (bias+scale on ACT) ------------------
        def q_proj(sc):
            xq_t = xpool.tile([128, KC, SQC], bf16, tag="xt", name=f"xq_{sc}")
            nc.sync.dma_start(xq_t[:], xqT[sc])
            for ft in range(2):
                ps = psP.tile([128, SQC], f32, tag="pp", name=f"psq_{sc}_{ft}")
                for kc in range(KC):
                    nc.tensor.matmul(
                        ps[:],
                        wq_sb[:, kc, ft * 128 : (ft + 1) * 128],
                        xq_t[:, kc, :],
                        start=(kc == 0),
                        stop=(kc == KC - 1),
                    )
                # out = in*0.125 + bq (bq pre-scaled by 0.125 on host)
                nc.scalar.activation(
                    QT_sb[:, ft, sc * SQC : (sc + 1) * SQC],
                    ps[:],
                    Ident,
                    bias=bq_sb[:, ft : ft + 1],
                    scale=0.125,
                )

        def v_proj(sc):
            # V projection (bias add on DVE via replicated-bias trick)
            xv_t = xpool.tile([128, KC, SQC], bf16, tag="xt", name=f"xv_{sc}")
            nc.sync.dma_start(xv_t[:], xvT[sc])
            for st in range(SQC // 128):
                t = sc * (SQC // 128) + st
                ps = psP.tile([128, F], f32, tag="pp", name=f"psv_{sc}_{st}")
                for kc in range(KC):
                    nc.tensor.matmul(
                        ps[:],
                        xv_t[:, kc, st * 128 : (st + 1) * 128],
                        wv_sb[:, kc, :],
                        start=(kc == 0),
                        stop=(kc == KC - 1),
                    )
                nc.vector.tensor_add(
                    V4[:, t, :, 0:D_HEAD],
                    ps[:].rearrange("p (h j) -> p h j", h=HEADS_PER_CORE),
                    bvRep.rearrange("p (h j) -> p h j", h=HEADS_PER_CORE),
                )

        def attn_scores(sc, h, m_t):
            # scores.T * mask -> exp : probs tiles [128 sk, NT, SQC] bf16
            ft = h // 2
            probs = ppool.tile([128, NT, SQC], bf16, tag="probs", name=f"probs_{sc}_{h}")
            for g in range(NEG):
                t0 = g * EG
                t1 = min(NT, t0 + EG)
                pf = fpool.tile([128, EG, SQC], f32, tag="pf", name=f"pf_{sc}_{h}_{g}")
                for t in range(t0, t1):
                    ps = psC.tile([128, SQC], f32, tag="pss", name=f"pss_{sc}_{h}_{t}")
                    nc.tensor.matmul(
                        ps[:],
                        KTz[:, h, t * 128 : (t + 1) * 128],
                        QT_sb[:, ft, sc * SQC : (sc + 1) * SQC],
                        start=True,
                        stop=True,
                    )
                    nc.vector.tensor_mul(pf[:, t - t0, :], ps[:], m_t[:, t, :])
                nc.scalar.activation(probs[:, t0:t1, :], pf[:, 0 : t1 - t0, :], Exp)
            return probs

        def attn_av(sc, h, probs):
            # ctx.T = (V|1).T @ probs.T : V|1 stationary, probs moving
            pc = psD.tile([D1, SQC], f32, tag="psc", name=f"psc_{sc}_{h}")
            for t in range(NT):
                nc.tensor.matmul(
                    pc[:],
                    V_sb[:, t, h * D1 : (h + 1) * D1],
                    probs[:, t, :],
                    start=(t == 0),
                    stop=(t == NT - 1),
                )
            co = copool.tile([D1, SQC], f32, tag="co", name=f"co_{sc}_{h}")
            nc.scalar.copy(co[:], pc[:])
            nc.sync.dma_start(out[h, :, sc * SQC : (sc + 1) * SQC], co[:])

        # Head 0 of chunk 0 starts as soon as K-proj + Q(0) + mask(0) are
        # ready; V projection overlaps its mask-multiplies on the DVE.
        q_proj(0)
        m0 = mpool.tile([128, NT, SQC], f32, tag="m", name="mask_0")
        nc.sync.dma_start(m0[:], maskT[0])
        probs00 = attn_scores(0, 0, m0)
        for sc in range(NSC):
            v_proj(sc)
        attn_av(0, 0, probs00)
        for h in range(1, HEADS_PER_CORE):
            p = attn_scores(0, h, m0)
            attn_av(0, h, p)
        for sc in range(1, NSC):
            q_proj(sc)
            m_t = mpool.tile([128, NT, SQC], f32, tag="m", name=f"mask_{sc}")
            nc.sync.dma_start(m_t[:], maskT[sc])
            for h in range(HEADS_PER_CORE):
                p = attn_scores(sc, h, m_t)
                attn_av(sc, h, p)

    nc.compile()
    return nc


def _x_slab(xb, S):
    """[S, D] input -> [NSC, 128, KC, SQC] bf16 per-chunk contiguous slab
    holding x.T tiles: slab[sc, p, c, j] = x[sc*SQC+j, c*128+p]."""
    SQC = min(512, S)
    NSC = S // SQC
    # x.T [D, S] -> (c p) (sc j)
    xt = xb.T.reshape(KC, 128, NSC, SQC)
    return np.ascontiguousarray(xt.transpose(2, 1, 0, 3)).astype(BF16)


def _w_slab(w):
    """[F, D] weight slice -> [128, KC, F] bf16: slab[p, c, f] = w[f, c*128+p]."""
    wt = w.T.reshape(KC, 128, F)
    return np.ascontiguousarray(wt.transpose(1, 0, 2)).astype(BF16)


def _mask_slab(mb, S):
    """[S, S] mask -> [NSC, 128, NT, SQC] f32 slab of mask.T tiles:
    slab[sc, p, t, j] = mask.T[t*128+p, sc*SQC+j] = mask[sc*SQC+j, t*128+p]."""
    SQC = min(512, S)
    NSC = S // SQC
    NT = S // 128
    mt = mb.T.reshape(NT, 128, NSC, SQC)
    return np.ascontiguousarray(mt.transpose(2, 1, 0, 3)).astype(np.float32)


def make_in_maps(q, k, v, mask, Wq, bq, Wk, bk, Wv, bv):
    """Build the 8 per-core input maps (host-side shard + transpose + cast)."""
    S = q.shape[1]
    xT = {}
    mT = {}
    for b in range(B):
        xT[("q", b)] = _x_slab(q[b], S)
        xT[("k", b)] = _x_slab(k[b], S)
        xT[("v", b)] = _x_slab(v[b], S)
        mT[b] = _mask_slab(mask[b, 0], S)

    in_maps = []
    for c in range(N_CORES):
        b = c // 4
        g = c % 4
        fs = slice(g * F, (g + 1) * F)
        in_maps.append(
            {
                "xqT": xT[("q", b)],
                "xkT": xT[("k", b)],
                "xvT": xT[("v", b)],
                "wqT": _w_slab(Wq[fs]),
                "wkT": _w_slab(Wk[fs]),
                "wvT": _w_slab(Wv[fs]),
                "bqs": np.ascontiguousarray(
                    (bq[fs] * 0.125).reshape(2, 128).T
                ).astype(np.float32),
                "bks": np.ascontiguousarray(bk[fs].reshape(2, 128).T).astype(
                    np.float32
                ),
                "bvr": np.ascontiguousarray(bv[fs].reshape(1, F)).astype(np.float32),
                "maskT": mT[b],
            }
        )
    return in_maps


def assemble_output(results, S=2048):
    ctx_all = np.empty((B, N_HEAD, S, D_HEAD), np.float32)
    for c in range(N_CORES):
        b = c // 4
        g = c % 4
        oc = results[c]["out"]  # [4, 65, S]: rows 0..63 ctx.T, row 64 denom
        for hl in range(HEADS_PER_CORE):
            ctx_all[b, g * 4 + hl] = (oc[hl, :D_HEAD, :] / oc[hl, D_HEAD:, :]).T
    return ctx_all.reshape(B, S, N_HEAD * D_HEAD)


def run_cores(in_maps, trace=False, **kwargs):
    if "nc" not in _NC_CACHE:
        _NC_CACHE["nc"] = build_nc()
    nc = _NC_CACHE["nc"]
    return run_bass_kernel_spmd(
        nc, in_maps, core_ids=list(range(N_CORES)), trace=trace, **kwargs
    )


def kernel(q, k, v, mask, Wq, bq, Wk, bk, Wv, bv):
    q = np.asarray(q, dtype=np.float32)
    k = np.asarray(k, dtype=np.float32)
    v = np.asarray(v, dtype=np.float32)
    mask = np.asarray(mask, dtype=np.float32)
    in_maps = make_in_maps(
        q,
        k,
        v,
        mask,
        np.asarray(Wq, np.float32),
        np.asarray(bq, np.float32),
        np.asarray(Wk, np.float32),
        np.asarray(bk, np.float32),
        np.asarray(Wv, np.float32),
        np.asarray(bv, np.float32),
    )
    res = run_cores(in_maps)
    return assemble_output(res.results, S=q.shape[1])


# revision 11
# speedup vs baseline: 1.0398x; 1.0398x over previous
"""Multi-head attention TRN2 Bass kernel.

Problem: B=2, S=2048, D_MODEL=1024, N_HEAD=16, D_HEAD=64.
  qh = split_heads(q @ Wq.T + bq) / 8;  kh, vh likewise (no scale)
  scores = (qh @ kh.T) * mask;  attn = softmax(scores);  ctx = attn @ vh
  out = ctx.reshape(B, S, 1024)   (raw [b,h,s,d] view, no head transpose-back)

Sharding (8 cores, tensor-parallel over heads + data-parallel over batch):
  core c -> batch b = c//4, heads [4*(c%4), 4*(c%4)+4)  (256 feature cols).

Per-core dataflow (all matmuls bf16 inputs, f32 PSUM accumulation):
  - host feeds x.T (bf16), W.T slices (bf16), mask.T (f32): no on-chip
    transposes needed anywhere.
  - Q.T in [dh, s] layout packed per head-pair; K.T per head zero-padded to
    128 contraction rows (K=64 matmuls stream the moving operand at half
    rate, so padding the stationary with zeros doubles throughput).
  - V in natural [s, f] layout with a ones column per head so the attn@V
    matmul also emits the softmax denominator.
  - scores.T tiles [sk=128, sq=512] via PE; mask multiply on DVE from PSUM
    into f32 staging; Exp on ACT into bf16 probs; ctx.T = (V|1).T @ probs.T
    accumulates over sk tiles with V|1 stationary (no per-tile weight
    reloads); PSUM->SBUF on ACT; normalization (divide by ones-column row)
    happens on the host after gather.
"""

import numpy as np
import ml_dtypes
from contextlib import ExitStack

import concourse.bass as bass
import concourse.mybir as mybir
import concourse.tile as tile
from concourse import bacc
from concourse.bass_utils import run_bass_kernel_spmd

BF16 = ml_dtypes.bfloat16

B = 2
D_MODEL = 1024
N_HEAD = 16
D_HEAD = 64
N_CORES = 8
HEADS_PER_CORE = 4
F = HEADS_PER_CORE * D_HEAD  # 256 feature cols per core
KC = D_MODEL // 128  # 8 contraction chunks

_NC_CACHE = {}


def build_nc(S=2048):
    f32 = mybir.dt.float32
    bf16 = mybir.dt.bfloat16
    Ident = mybir.ActivationFunctionType.Identity
    Exp = mybir.ActivationFunctionType.Exp

    SQC = min(512, S)        # sq chunk (one scores matmul N)
    NSC = S // SQC           # number of sq chunks
    NT = S // 128            # number of sk tiles
    EG = 4                   # sk tiles per Exp batch
    NEG = (NT + EG - 1) // EG
    D1 = D_HEAD + 1          # head cols incl. denominator ones column

    nc = bacc.Bacc("TRN2", target_bir_lowering=False, debug=False)

    xqT = nc.dram_tensor("xqT", [NSC, 128, KC, SQC], bf16, kind="ExternalInput").ap()
    xkT = nc.dram_tensor("xkT", [NSC, 128, KC, SQC], bf16, kind="ExternalInput").ap()
    xvT = nc.dram_tensor("xvT", [NSC, 128, KC, SQC], bf16, kind="ExternalInput").ap()
    wqT = nc.dram_tensor("wqT", [128, KC, F], bf16, kind="ExternalInput").ap()
    wkT = nc.dram_tensor("wkT", [128, KC, F], bf16, kind="ExternalInput").ap()
    wvT = nc.dram_tensor("wvT", [128, KC, F], bf16, kind="ExternalInput").ap()
    bqs = nc.dram_tensor("bqs", [128, 2], f32, kind="ExternalInput").ap()
    bks = nc.dram_tensor("bks", [128, 2], f32, kind="ExternalInput").ap()
    bvr = nc.dram_tensor("bvr", [1, F], f32, kind="ExternalInput").ap()
    maskT = nc.dram_tensor("maskT", [NSC, 128, NT, SQC], bf16, kind="ExternalInput").ap()
    # per-head transposed context incl. denominator row (normalized on host)
    out = nc.dram_tensor(
        "out", [HEADS_PER_CORE, D1, S], f32, kind="ExternalOutput"
    ).ap()


    with tile.TileContext(nc) as tc, ExitStack() as ctx:
        const = ctx.enter_context(tc.tile_pool(name="const", bufs=1))
        persist = ctx.enter_context(tc.tile_pool(name="persist", bufs=1))
        xpool = ctx.enter_context(tc.tile_pool(name="xpool", bufs=2))
        mpool = ctx.enter_context(tc.tile_pool(name="mpool", bufs=2))
        fpool = ctx.enter_context(tc.tile_pool(name="fpool", bufs=2))
        ppool = ctx.enter_context(tc.tile_pool(name="ppool", bufs=2))
        copool = ctx.enter_context(tc.tile_pool(name="copool", bufs=3))
        # PSUM: 5 shared (proj + scores) + 3 (ctx) = 8 banks
        psP = ctx.enter_context(tc.tile_pool(name="psP", bufs=5, space="PSUM"))
        psC = psP
        psD = ctx.enter_context(tc.tile_pool(name="psD", bufs=3, space="PSUM"))

        # --- constants -----------------------------------------------------
        wq_sb = const.tile([128, KC, F], bf16)
        nc.sync.dma_start(wq_sb[:], wqT)
        wk_sb = const.tile([128, KC, F], bf16)
        nc.sync.dma_start(wk_sb[:], wkT)
        wv_sb = const.tile([128, KC, F], bf16)
        nc.sync.dma_start(wv_sb[:], wvT)
        bq_sb = const.tile([128, 2], f32)
        nc.sync.dma_start(bq_sb[:], bqs)
        bk_sb = const.tile([128, 2], f32)
        nc.sync.dma_start(bk_sb[:], bks)
        bv_sb = const.tile([1, F], f32)
        nc.sync.dma_start(bv_sb[:], bvr)

        ones1 = const.tile([1, 128], f32)
        nc.vector.memset(ones1[:], 1.0)
        bvp = psP.tile([128, F], f32, tag="pp")
        nc.tensor.matmul(bvp[:], ones1[:], bv_sb[:], start=True, stop=True)
        bvRep = const.tile([128, F], f32)
        nc.vector.tensor_copy(bvRep[:], bvp[:])

        QT_sb = persist.tile([128, 2, S], bf16)
        # K.T zero-padded per head: head h lives in rows (h%2)*64..+64 of
        # KTz[:, h, :], the other 64 rows stay zero -> scores run at K=128.
        KTz = persist.tile([128, HEADS_PER_CORE, S], bf16)
        for h in range(HEADS_PER_CORE):
            zr = (1 - h % 2) * 64
            nc.gpsimd.memset(KTz[zr : zr + 64, h, :], 0.0)
        V_sb = persist.tile([128, NT, HEADS_PER_CORE * D1], bf16)
        V4 = V_sb.rearrange("p t (h j) -> p t h j", h=HEADS_PER_CORE)
        nc.gpsimd.memset(V4[:, :, :, D_HEAD : D_HEAD + 1], 1.0)

        # --- K projection (+ head-split into KTz halves, bias on ACT) ------
        for sc in range(NSC):
            xk_t = xpool.tile([128, KC, SQC], bf16, tag="xk", bufs=4)
            nc.sync.dma_start(xk_t[:], xkT[sc])
            for ft in range(2):
                ps = psP.tile([128, SQC], f32, tag="pp")
                for kc in range(KC):
                    nc.tensor.matmul(
                        ps[:],
                        wk_sb[:, kc, ft * 128 : (ft + 1) * 128],
                        xk_t[:, kc, :],
                        start=(kc == 0),
                        stop=(kc == KC - 1),
                    )
                sl = slice(sc * SQC, (sc + 1) * SQC)
                nc.scalar.activation(
                    KTz[0:64, 2 * ft, sl], ps[0:64, :], Ident, bias=bk_sb[0:64, ft : ft + 1]
                )
                nc.scalar.activation(
                    KTz[64:128, 2 * ft + 1, sl],
                    ps[64:128, :],
                    Ident,
                    bias=bk_sb[64:128, ft : ft + 1],
                )

        # --- Q projection chunk maker (bias+scale on ACT) ------------------
        def q_proj(sc):
            xq_t = xpool.tile([128, KC, SQC], bf16, tag="xt", name=f"xq_{sc}")
            nc.sync.dma_start(xq_t[:], xqT[sc])
            for ft in range(2):
                ps = psP.tile([128, SQC], f32, tag="pp", name=f"psq_{sc}_{ft}")
                for kc in range(KC):
                    nc.tensor.matmul(
                        ps[:],
                        wq_sb[:, kc, ft * 128 : (ft + 1) * 128],
                        xq_t[:, kc, :],
                        start=(kc == 0),
                        stop=(kc == KC - 1),
                    )
                # out = in*0.125 + bq (bq pre-scaled by 0.125 on host)
                nc.scalar.activation(
                    QT_sb[:, ft, sc * SQC : (sc + 1) * SQC],
                    ps[:],
                    Ident,
                    bias=bq_sb[:, ft : ft + 1],
                    scale=0.125,
                )

        def v_proj(sc):
            # V projection (bias add on DVE via replicated-bias trick)
            xv_t = xpool.tile([128, KC, SQC], bf16, tag="xt", name=f"xv_{sc}")
            nc.sync.dma_start(xv_t[:], xvT[sc])
            for st in range(SQC // 128):
                t = sc * (SQC // 128) + st
                ps = psP.tile([128, F], f32, tag="pp", name=f"psv_{sc}_{st}")
                for kc in range(KC):
                    nc.tensor.matmul(
                        ps[:],
                        xv_t[:, kc, st * 128 : (st + 1) * 128],
                        wv_sb[:, kc, :],
                        start=(kc == 0),
                        stop=(kc == KC - 1),
                    )
                nc.vector.tensor_add(
                    V4[:, t, :, 0:D_HEAD],
                    ps[:].rearrange("p (h j) -> p h j", h=HEADS_PER_CORE),
                    bvRep.rearrange("p (h j) -> p h j", h=HEADS_PER_CORE),
                )

        def attn_scores(sc, h, m_t):
            # scores.T * mask -> exp : probs tiles [128 sk, NT, SQC] bf16
            ft = h // 2
            probs = ppool.tile([128, NT, SQC], bf16, tag="probs", name=f"probs_{sc}_{h}")
            for g in range(NEG):
                t0 = g * EG
                t1 = min(NT, t0 + EG)
                pf = fpool.tile([128, EG, SQC], f32, tag="pf", name=f"pf_{sc}_{h}_{g}")
                for t in range(t0, t1):
                    ps = psC.tile([128, SQC], f32, tag="pp", name=f"pss_{sc}_{h}_{t}")
                    nc.tensor.matmul(
                        ps[:],
                        KTz[:, h, t * 128 : (t + 1) * 128],
                        QT_sb[:, ft, sc * SQC : (sc + 1) * SQC],
                        start=True,
                        stop=True,
                    )
                    nc.vector.tensor_mul(pf[:, t - t0, :], ps[:], m_t[:, t, :])
                nc.scalar.activation(probs[:, t0:t1, :], pf[:, 0 : t1 - t0, :], Exp)
            return probs

        def attn_av(sc, h, probs):
            # ctx.T = (V|1).T @ probs.T : V|1 stationary, probs moving
            pc = psD.tile([D1, SQC], f32, tag="psc", name=f"psc_{sc}_{h}")
            for t in range(NT):
                nc.tensor.matmul(
                    pc[:],
                    V_sb[:, t, h * D1 : (h + 1) * D1],
                    probs[:, t, :],
                    start=(t == 0),
                    stop=(t == NT - 1),
                )
            co = copool.tile([D1, SQC], f32, tag="co", name=f"co_{sc}_{h}")
            nc.scalar.copy(co[:], pc[:])
            nc.sync.dma_start(out[h, :, sc * SQC : (sc + 1) * SQC], co[:])

        # Head 0 of chunk 0 starts as soon as K-proj + Q(0) + mask(0) are
        # ready; V projection overlaps its mask-multiplies on the DVE.
        q_proj(0)
        m0 = mpool.tile([128, NT, SQC], bf16, tag="m", name="mask_0")
        nc.sync.dma_start(m0[:], maskT[0])
        probs00 = attn_scores(0, 0, m0)
        for sc in range(NSC):
            v_proj(sc)
        attn_av(0, 0, probs00)
        for h in range(1, HEADS_PER_CORE):
            p = attn_scores(0, h, m0)
            attn_av(0, h, p)
        for sc in range(1, NSC):
            q_proj(sc)
            m_t = mpool.tile([128, NT, SQC], bf16, tag="m", name=f"mask_{sc}")
            nc.sync.dma_start(m_t[:], maskT[sc])
            for h in range(HEADS_PER_CORE):
                p = attn_scores(sc, h, m_t)
                attn_av(sc, h, p)

    nc.compile()
    return nc


def _x_slab(xb, S):
    """[S, D] input -> [NSC, 128, KC, SQC] bf16 per-chunk contiguous slab
    holding x.T tiles: slab[sc, p, c, j] = x[sc*SQC+j, c*128+p]."""
    SQC = min(512, S)
    NSC = S // SQC
    # x.T [D, S] -> (c p) (sc j)
    xt = xb.T.reshape(KC, 128, NSC, SQC)
    return np.ascontiguousarray(xt.transpose(2, 1, 0, 3)).astype(BF16)


def _w_slab(w):
    """[F, D] weight slice -> [128, KC, F] bf16: slab[p, c, f] = w[f, c*128+p]."""
    wt = w.T.reshape(KC, 128, F)
    return np.ascontiguousarray(wt.transpose(1, 0, 2)).astype(BF16)


def _mask_slab(mb, S):
    """[S, S] mask -> [NSC, 128, NT, SQC] f32 slab of mask.T tiles:
    slab[sc, p, t, j] = mask.T[t*128+p, sc*SQC+j] = mask[sc*SQC+j, t*128+p]."""
    SQC = min(512, S)
    NSC = S // SQC
    NT = S // 128
    mt = mb.T.reshape(NT, 128, NSC, SQC)
    return np.ascontiguousarray(mt.transpose(2, 1, 0, 3)).astype(BF16)


def make_in_maps(q, k, v, mask, Wq, bq, Wk, bk, Wv, bv):
    """Build the 8 per-core input maps (host-side shard + transpose + cast)."""
    S = q.shape[1]
    xT = {}
    mT = {}
    for b in range(B):
        xT[("q", b)] = _x_slab(q[b], S)
        xT[("k", b)] = _x_slab(k[b], S)
        xT[("v", b)] = _x_slab(v[b], S)
        mT[b] = _mask_slab(mask[b, 0], S)

    in_maps = []
    for c in range(N_CORES):
        b = c // 4
        g = c % 4
        fs = slice(g * F, (g + 1) * F)
        in_maps.append(
            {
                "xqT": xT[("q", b)],
                "xkT": xT[("k", b)],
                "xvT": xT[("v", b)],
                "wqT": _w_slab(Wq[fs]),
                "wkT": _w_slab(Wk[fs]),
                "wvT": _w_slab(Wv[fs]),
                "bqs": np.ascontiguousarray(
                    (bq[fs] * 0.125).reshape(2, 128).T
                ).astype(np.float32),
                "bks": np.ascontiguousarray(bk[fs].reshape(2, 128).T).astype(
                    np.float32
                ),
                "bvr": np.ascontiguousarray(bv[fs].reshape(1, F)).astype(np.float32),
                "maskT": mT[b],
            }
        )
    return in_maps


def assemble_output(results, S=2048):
    ctx_all = np.empty((B, N_HEAD, S, D_HEAD), np.float32)
    for c in range(N_CORES):
        b = c // 4
        g = c % 4
        oc = results[c]["out"]  # [4, 65, S]: rows 0..63 ctx.T, row 64 denom
        for hl in range(HEADS_PER_CORE):
            ctx_all[b, g * 4 + hl] = (oc[hl, :D_HEAD, :] / oc[hl, D_HEAD:, :]).T
    return ctx_all.reshape(B, S, N_HEAD * D_HEAD)


def run_cores(in_maps, trace=False, **kwargs):
    if "nc" not in _NC_CACHE:
        _NC_CACHE["nc"] = build_nc()
    nc = _NC_CACHE["nc"]
    return run_bass_kernel_spmd(
        nc, in_maps, core_ids=list(range(N_CORES)), trace=trace, **kwargs
    )


def kernel(q, k, v, mask, Wq, bq, Wk, bk, Wv, bv):
    q = np.asarray(q, dtype=np.float32)
    k = np.asarray(k, dtype=np.float32)
    v = np.asarray(v, dtype=np.float32)
    mask = np.asarray(mask, dtype=np.float32)
    in_maps = make_in_maps(
        q,
        k,
        v,
        mask,
        np.asarray(Wq, np.float32),
        np.asarray(bq, np.float32),
        np.asarray(Wk, np.float32),
        np.asarray(bk, np.float32),
        np.asarray(Wv, np.float32),
        np.asarray(bv, np.float32),
    )
    res = run_cores(in_maps)
    return assemble_output(res.results, S=q.shape[1])


# revision 27
# speedup vs baseline: 1.0456x; 1.0056x over previous
"""Multi-head attention TRN2 Bass kernel.

Problem: B=2, S=2048, D_MODEL=1024, N_HEAD=16, D_HEAD=64.
  qh = split_heads(q @ Wq.T + bq) / 8;  kh, vh likewise (no scale)
  scores = (qh @ kh.T) * mask;  attn = softmax(scores);  ctx = attn @ vh
  out = ctx.reshape(B, S, 1024)   (raw [b,h,s,d] view, no head transpose-back)

Sharding (8 cores, tensor-parallel over heads + data-parallel over batch):
  core c -> batch b = c//4, heads [4*(c%4), 4*(c%4)+4)  (256 feature cols).

Per-core dataflow (all matmuls bf16 inputs, f32 PSUM accumulation):
  - host feeds x.T (bf16), W.T slices (bf16), mask.T (f32): no on-chip
    transposes needed anywhere.
  - Q.T in [dh, s] layout packed per head-pair; K.T per head zero-padded to
    128 contraction rows (K=64 matmuls stream the moving operand at half
    rate, so padding the stationary with zeros doubles throughput).
  - V in natural [s, f] layout with a ones column per head so the attn@V
    matmul also emits the softmax denominator.
  - scores.T tiles [sk=128, sq=512] via PE; mask multiply on DVE from PSUM
    into f32 staging; Exp on ACT into bf16 probs; ctx.T = (V|1).T @ probs.T
    accumulates over sk tiles with V|1 stationary (no per-tile weight
    reloads); PSUM->SBUF on ACT; normalization (divide by ones-column row)
    happens on the host after gather.
"""

import numpy as np
import ml_dtypes
from contextlib import ExitStack

import concourse.bass as bass
import concourse.mybir as mybir
import concourse.tile as tile
from concourse import bacc
from concourse.bass_utils import run_bass_kernel_spmd

BF16 = ml_dtypes.bfloat16

B = 2
D_MODEL = 1024
N_HEAD = 16
D_HEAD = 64
N_CORES = 8
HEADS_PER_CORE = 4
F = HEADS_PER_CORE * D_HEAD  # 256 feature cols per core
KC = D_MODEL // 128  # 8 contraction chunks

_NC_CACHE = {}


def build_nc(S=2048):
    f32 = mybir.dt.float32
    bf16 = mybir.dt.bfloat16
    Ident = mybir.ActivationFunctionType.Identity
    Exp = mybir.ActivationFunctionType.Exp

    SQC = min(512, S)        # sq chunk (one scores matmul N)
    NSC = S // SQC           # number of sq chunks
    NT = S // 128            # number of sk tiles
    EG = 8                   # sk tiles per Exp batch
    NEG = (NT + EG - 1) // EG
    D1 = D_HEAD + 1          # head cols incl. denominator ones column

    nc = bacc.Bacc("TRN2", target_bir_lowering=False, debug=False)

    xqT = nc.dram_tensor("xqT", [NSC, 128, KC, SQC], bf16, kind="ExternalInput").ap()
    xkT = nc.dram_tensor("xkT", [NSC, 128, KC, SQC], bf16, kind="ExternalInput").ap()
    xvT = nc.dram_tensor("xvT", [NSC, 128, KC, SQC], bf16, kind="ExternalInput").ap()
    wqT = nc.dram_tensor("wqT", [128, KC, F], bf16, kind="ExternalInput").ap()
    wkT = nc.dram_tensor("wkT", [128, KC, F], bf16, kind="ExternalInput").ap()
    wvT = nc.dram_tensor("wvT", [128, KC, F], bf16, kind="ExternalInput").ap()
    bqs = nc.dram_tensor("bqs", [128, 2], f32, kind="ExternalInput").ap()
    bks = nc.dram_tensor("bks", [128, 2], f32, kind="ExternalInput").ap()
    bvr = nc.dram_tensor("bvr", [1, F], f32, kind="ExternalInput").ap()
    maskT = nc.dram_tensor("maskT", [NSC, 128, NT, SQC], bf16, kind="ExternalInput").ap()
    # per-head transposed context incl. denominator row (normalized on host)
    out = nc.dram_tensor(
        "out", [HEADS_PER_CORE, D1, S], f32, kind="ExternalOutput"
    ).ap()


    with tile.TileContext(nc) as tc, ExitStack() as ctx:
        const = ctx.enter_context(tc.tile_pool(name="const", bufs=1))
        persist = ctx.enter_context(tc.tile_pool(name="persist", bufs=1))
        xpool = ctx.enter_context(tc.tile_pool(name="xpool", bufs=2))
        mpool = ctx.enter_context(tc.tile_pool(name="mpool", bufs=2))
        fpool = ctx.enter_context(tc.tile_pool(name="fpool", bufs=2))
        ppool = ctx.enter_context(tc.tile_pool(name="ppool", bufs=2))
        copool = ctx.enter_context(tc.tile_pool(name="copool", bufs=2))
        spool = ctx.enter_context(tc.tile_pool(name="spool", bufs=3))
        # PSUM: 5 shared (proj + scores) + 3 (ctx) = 8 banks
        psP = ctx.enter_context(tc.tile_pool(name="psP", bufs=5, space="PSUM"))
        psC = psP
        psD = ctx.enter_context(tc.tile_pool(name="psD", bufs=3, space="PSUM"))

        # --- constants (DMA in critical-path order: K-proj needs wk first,
        # then Q0, then mask chunk 0; V-path items come after) --------------
        wk_sb = const.tile([128, KC, F], bf16)
        nc.sync.dma_start(wk_sb[:], wkT)
        wq_sb = const.tile([128, KC, F], bf16)
        nc.sync.dma_start(wq_sb[:], wqT)
        bk_sb = const.tile([128, 2], f32)
        nc.sync.dma_start(bk_sb[:], bks)
        bq_sb = const.tile([128, 2], f32)
        nc.sync.dma_start(bq_sb[:], bqs)
        m0 = mpool.tile([128, NT, SQC], bf16, tag="m", name="mask_0")
        wv_sb = const.tile([128, KC, F], bf16)
        bv_sb = const.tile([1, F], f32)

        ones1 = const.tile([1, 128], f32)
        nc.vector.memset(ones1[:], 1.0)
        bvRep = const.tile([128, F], f32)

        QT_sb = persist.tile([128, 2, S], bf16)
        # K.T zero-padded per head: head h lives in rows (h%2)*64..+64 of
        # KTz[:, h, :], the other 64 rows stay zero -> scores run at K=128.
        KTz = persist.tile([128, HEADS_PER_CORE, S], bf16)
        for h in range(HEADS_PER_CORE):
            zr = (1 - h % 2) * 64
            nc.gpsimd.memset(KTz[zr : zr + 64, h, :], 0.0)
        V_sb = persist.tile([128, NT, HEADS_PER_CORE * D1], bf16)
        V4 = V_sb.rearrange("p t (h j) -> p t h j", h=HEADS_PER_CORE)
        nc.gpsimd.memset(V4[:, :, :, D_HEAD : D_HEAD + 1], 1.0)

        # --- K projection (+ head-split into KTz halves, bias on ACT) ------
        for sc in range(NSC):
            xk_t = xpool.tile([128, KC, SQC], bf16, tag="xk", bufs=3)
            nc.sync.dma_start(xk_t[:], xkT[sc])
            for ft in range(2):
                ps = psP.tile([128, SQC], f32, tag="pp")
                for kc in range(KC):
                    nc.tensor.matmul(
                        ps[:],
                        wk_sb[:, kc, ft * 128 : (ft + 1) * 128],
                        xk_t[:, kc, :],
                        start=(kc == 0),
                        stop=(kc == KC - 1),
                    )
                sl = slice(sc * SQC, (sc + 1) * SQC)
                nc.scalar.activation(
                    KTz[0:64, 2 * ft, sl], ps[0:64, :], Ident, bias=bk_sb[0:64, ft : ft + 1]
                )
                nc.scalar.activation(
                    KTz[64:128, 2 * ft + 1, sl],
                    ps[64:128, :],
                    Ident,
                    bias=bk_sb[64:128, ft : ft + 1],
                )

        # --- Q projection chunk maker (bias+scale on ACT) ------------------
        def q_load(sc):
            xq_t = xpool.tile([128, KC, SQC], bf16, tag="xt", name=f"xq_{sc}")
            nc.sync.dma_start(xq_t[:], xqT[sc])
            return xq_t

        def q_proj(sc, xq_t=None):
            if xq_t is None:
                xq_t = q_load(sc)
            for ft in range(2):
                ps = psP.tile([128, SQC], f32, tag="pp", name=f"psq_{sc}_{ft}")
                for kc in range(KC):
                    nc.tensor.matmul(
                        ps[:],
                        wq_sb[:, kc, ft * 128 : (ft + 1) * 128],
                        xq_t[:, kc, :],
                        start=(kc == 0),
                        stop=(kc == KC - 1),
                    )
                # out = in*0.125 + bq (bq pre-scaled by 0.125 on host)
                nc.scalar.activation(
                    QT_sb[:, ft, sc * SQC : (sc + 1) * SQC],
                    ps[:],
                    Ident,
                    bias=bq_sb[:, ft : ft + 1],
                    scale=0.125,
                )

        def v_proj(sc):
            # V projection (bias add on DVE via replicated-bias trick)
            xv_t = xpool.tile([128, KC, SQC], bf16, tag="xt", name=f"xv_{sc}")
            nc.sync.dma_start(xv_t[:], xvT[sc])
            for st in range(SQC // 128):
                t = sc * (SQC // 128) + st
                ps = psP.tile([128, F], f32, tag="pp", name=f"psv_{sc}_{st}")
                for kc in range(KC):
                    nc.tensor.matmul(
                        ps[:],
                        xv_t[:, kc, st * 128 : (st + 1) * 128],
                        wv_sb[:, kc, :],
                        start=(kc == 0),
                        stop=(kc == KC - 1),
                    )
                nc.vector.tensor_add(
                    V4[:, t, :, 0:D_HEAD],
                    ps[:].rearrange("p (h j) -> p h j", h=HEADS_PER_CORE),
                    bvRep.rearrange("p (h j) -> p h j", h=HEADS_PER_CORE),
                )

        def attn_scores(sc, h, m_t):
            # scores.T * mask -> exp : probs tiles [128 sk, NT, SQC] bf16.
            # A few tiles per head detour via ACT-copy + GpSimd multiply to
            # offload the DVE (the overall bottleneck).
            ft = h // 2
            gp_tiles = {5}
            probs = ppool.tile([128, NT, SQC], bf16, tag="probs", name=f"probs_{sc}_{h}")
            for g in range(NEG):
                t0 = g * EG
                t1 = min(NT, t0 + EG)
                pf = fpool.tile([128, EG, SQC], f32, tag="pf", name=f"pf_{sc}_{h}_{g}")
                for t in range(t0, t1):
                    ps = psC.tile([128, SQC], f32, tag="pp", name=f"pss_{sc}_{h}_{t}")
                    nc.tensor.matmul(
                        ps[:],
                        KTz[:, h, t * 128 : (t + 1) * 128],
                        QT_sb[:, ft, sc * SQC : (sc + 1) * SQC],
                        start=True,
                        stop=True,
                    )
                    if t in gp_tiles:
                        ss = spool.tile([128, SQC], bf16, tag="ss", name=f"ss_{sc}_{h}_{t}")
                        nc.scalar.copy(ss[:], ps[:])
                        nc.gpsimd.tensor_mul(pf[:, t - t0, :], ss[:], m_t[:, t, :])
                    else:
                        nc.vector.tensor_mul(pf[:, t - t0, :], ps[:], m_t[:, t, :])
                nc.scalar.activation(probs[:, t0:t1, :], pf[:, 0 : t1 - t0, :], Exp)
            return probs

        def attn_av(sc, h, probs):
            # ctx.T = (V|1).T @ probs.T : V|1 stationary, probs moving
            pc = psD.tile([D1, SQC], f32, tag="psc", name=f"psc_{sc}_{h}")
            for t in range(NT):
                nc.tensor.matmul(
                    pc[:],
                    V_sb[:, t, h * D1 : (h + 1) * D1],
                    probs[:, t, :],
                    start=(t == 0),
                    stop=(t == NT - 1),
                )
            co = copool.tile([D1, SQC], f32, tag="co", name=f"co_{sc}_{h}")
            nc.scalar.copy(co[:], pc[:])
            nc.sync.dma_start(out[h, :, sc * SQC : (sc + 1) * SQC], co[:])

        # Head 0 of chunk 0 starts as soon as K-proj + Q(0) + mask(0) are
        # ready; V projection overlaps its mask-multiplies on the DVE, and
        # each AV matmul is software-pipelined one head behind its scores so
        # the PE never stalls the DVE at head/chunk boundaries.
        q_proj(0)
        nc.sync.dma_start(m0[:], maskT[0])
        nc.sync.dma_start(wv_sb[:], wvT)
        nc.sync.dma_start(bv_sb[:], bvr)
        bvp = psP.tile([128, F], f32, tag="pp")
        nc.tensor.matmul(bvp[:], ones1[:], bv_sb[:], start=True, stop=True)
        nc.vector.tensor_copy(bvRep[:], bvp[:])
        probs00 = attn_scores(0, 0, m0)
        for sc in range(NSC):
            v_proj(sc)
        pend = (0, 0, probs00)
        masks = {0: m0}
        for sc in range(NSC):
            if sc > 0:
                q_proj(sc)
            m_t = masks[sc]
            for h in range(HEADS_PER_CORE):
                if sc == 0 and h == 0:
                    continue
                if h == 1 and sc + 1 < NSC:
                    # prefetch next chunk's mask while this chunk computes
                    mn = mpool.tile([128, NT, SQC], bf16, tag="m", name=f"mask_{sc+1}")
                    nc.sync.dma_start(mn[:], maskT[sc + 1])
                    masks[sc + 1] = mn
                p = attn_scores(sc, h, m_t)
                attn_av(*pend)
                pend = (sc, h, p)
        attn_av(*pend)

    nc.compile()
    return nc


def _x_slab(xb, S):
    """[S, D] input -> [NSC, 128, KC, SQC] bf16 per-chunk contiguous slab
    holding x.T tiles: slab[sc, p, c, j] = x[sc*SQC+j, c*128+p]."""
    SQC = min(512, S)
    NSC = S // SQC
    # x.T [D, S] -> (c p) (sc j)
    xt = xb.T.reshape(KC, 128, NSC, SQC)
    return np.ascontiguousarray(xt.transpose(2, 1, 0, 3)).astype(BF16)


def _w_slab(w):
    """[F, D] weight slice -> [128, KC, F] bf16: slab[p, c, f] = w[f, c*128+p]."""
    wt = w.T.reshape(KC, 128, F)
    return np.ascontiguousarray(wt.transpose(1, 0, 2)).astype(BF16)


def _mask_slab(mb, S):
    """[S, S] mask -> [NSC, 128, NT, SQC] f32 slab of mask.T tiles:
    slab[sc, p, t, j] = mask.T[t*128+p, sc*SQC+j] = mask[sc*SQC+j, t*128+p]."""
    SQC = min(512, S)
    NSC = S // SQC
    NT = S // 128
    mt = mb.T.reshape(NT, 128, NSC, SQC)
    return np.ascontiguousarray(mt.transpose(2, 1, 0, 3)).astype(BF16)


def make_in_maps(q, k, v, mask, Wq, bq, Wk, bk, Wv, bv):
    """Build the 8 per-core input maps (host-side shard + transpose + cast)."""
    S = q.shape[1]
    xT = {}
    mT = {}
    for b in range(B):
        xT[("q", b)] = _x_slab(q[b], S)
        xT[("k", b)] = _x_slab(k[b], S)
        xT[("v", b)] = _x_slab(v[b], S)
        mT[b] = _mask_slab(mask[b, 0], S)

    in_maps = []
    for c in range(N_CORES):
        b = c // 4
        g = c % 4
        fs = slice(g * F, (g + 1) * F)
        in_maps.append(
            {
                "xqT": xT[("q", b)],
                "xkT": xT[("k", b)],
                "xvT": xT[("v", b)],
                "wqT": _w_slab(Wq[fs]),
                "wkT": _w_slab(Wk[fs]),
                "wvT": _w_slab(Wv[fs]),
                "bqs": np.ascontiguousarray(
                    (bq[fs] * 0.125).reshape(2, 128).T
                ).astype(np.float32),
                "bks": np.ascontiguousarray(bk[fs].reshape(2, 128).T).astype(
                    np.float32
                ),
                "bvr": np.ascontiguousarray(bv[fs].reshape(1, F)).astype(np.float32),
                "maskT": mT[b],
            }
        )
    return in_maps


def assemble_output(results, S=2048):
    ctx_all = np.empty((B, N_HEAD, S, D_HEAD), np.float32)
    for c in range(N_CORES):
        b = c // 4
        g = c % 4
        oc = results[c]["out"]  # [4, 65, S]: rows 0..63 ctx.T, row 64 denom
        for hl in range(HEADS_PER_CORE):
            ctx_all[b, g * 4 + hl] = (oc[hl, :D_HEAD, :] / oc[hl, D_HEAD:, :]).T
    return ctx_all.reshape(B, S, N_HEAD * D_HEAD)


def run_cores(in_maps, trace=False, **kwargs):
    if "nc" not in _NC_CACHE:
        _NC_CACHE["nc"] = build_nc()
    nc = _NC_CACHE["nc"]
    return run_bass_kernel_spmd(
        nc, in_maps, core_ids=list(range(N_CORES)), trace=trace, **kwargs
    )


def kernel(q, k, v, mask, Wq, bq, Wk, bk, Wv, bv):
    q = np.asarray(q, dtype=np.float32)
    k = np.asarray(k, dtype=np.float32)
    v = np.asarray(v, dtype=np.float32)
    mask = np.asarray(mask, dtype=np.float32)
    in_maps = make_in_maps(
        q,
        k,
        v,
        mask,
        np.asarray(Wq, np.float32),
        np.asarray(bq, np.float32),
        np.asarray(Wk, np.float32),
        np.asarray(bk, np.float32),
        np.asarray(Wv, np.float32),
        np.asarray(bv, np.float32),
    )
    res = run_cores(in_maps)
    return assemble_output(res.results, S=q.shape[1])


# revision 30
# speedup vs baseline: 1.0749x; 1.0281x over previous
"""Multi-head attention TRN2 Bass kernel.

Problem: B=2, S=2048, D_MODEL=1024, N_HEAD=16, D_HEAD=64.
  qh = split_heads(q @ Wq.T + bq) / 8;  kh, vh likewise (no scale)
  scores = (qh @ kh.T) * mask;  attn = softmax(scores);  ctx = attn @ vh
  out = ctx.reshape(B, S, 1024)   (raw [b,h,s,d] view, no head transpose-back)

Sharding (8 cores, tensor-parallel over heads + data-parallel over batch):
  core c -> batch b = c//4, heads [4*(c%4), 4*(c%4)+4)  (256 feature cols).

Per-core dataflow (all matmuls bf16 inputs, f32 PSUM accumulation):
  - host feeds x.T (bf16), W.T slices (bf16), mask.T (f32): no on-chip
    transposes needed anywhere.
  - Q.T in [dh, s] layout packed per head-pair; K.T per head zero-padded to
    128 contraction rows (K=64 matmuls stream the moving operand at half
    rate, so padding the stationary with zeros doubles throughput).
  - V in natural [s, f] layout with a ones column per head so the attn@V
    matmul also emits the softmax denominator.
  - scores.T tiles [sk=128, sq=512] via PE; mask multiply on DVE from PSUM
    into f32 staging; Exp on ACT into bf16 probs; ctx.T = (V|1).T @ probs.T
    accumulates over sk tiles with V|1 stationary (no per-tile weight
    reloads); PSUM->SBUF on ACT; normalization (divide by ones-column row)
    happens on the host after gather.
"""

import numpy as np
import ml_dtypes
from contextlib import ExitStack

import concourse.bass as bass
import concourse.mybir as mybir
import concourse.tile as tile
from concourse import bacc
from concourse.bass_utils import run_bass_kernel_spmd

BF16 = ml_dtypes.bfloat16

B = 2
D_MODEL = 1024
N_HEAD = 16
D_HEAD = 64
N_CORES = 8
HEADS_PER_CORE = 4
F = HEADS_PER_CORE * D_HEAD  # 256 feature cols per core
KC = D_MODEL // 128  # 8 contraction chunks

_NC_CACHE = {}


def build_nc(S=2048):
    f32 = mybir.dt.float32
    bf16 = mybir.dt.bfloat16
    Ident = mybir.ActivationFunctionType.Identity
    Exp = mybir.ActivationFunctionType.Exp

    SQC = min(512, S)        # sq chunk (one scores matmul N)
    NSC = S // SQC           # number of sq chunks
    NT = S // 128            # number of sk tiles
    EG = 8                   # sk tiles per Exp batch
    NEG = (NT + EG - 1) // EG
    D1 = D_HEAD + 1          # head cols incl. denominator ones column

    nc = bacc.Bacc("TRN2", target_bir_lowering=False, debug=False)

    xqT = nc.dram_tensor("xqT", [NSC, 128, KC, SQC], bf16, kind="ExternalInput").ap()
    xkT = nc.dram_tensor("xkT", [NSC, 128, KC, SQC], bf16, kind="ExternalInput").ap()
    xvT = nc.dram_tensor("xvT", [NSC, 128, KC, SQC], bf16, kind="ExternalInput").ap()
    wqT = nc.dram_tensor("wqT", [128, KC, F], bf16, kind="ExternalInput").ap()
    wkT = nc.dram_tensor("wkT", [128, KC, F], bf16, kind="ExternalInput").ap()
    wvT = nc.dram_tensor("wvT", [128, KC, F], bf16, kind="ExternalInput").ap()
    bqs = nc.dram_tensor("bqs", [128, 2], f32, kind="ExternalInput").ap()
    bks = nc.dram_tensor("bks", [128, 2], f32, kind="ExternalInput").ap()
    bvr = nc.dram_tensor("bvr", [1, F], f32, kind="ExternalInput").ap()
    maskT = nc.dram_tensor("maskT", [NSC, 128, NT, SQC], bf16, kind="ExternalInput").ap()
    # per-head transposed context incl. denominator row (normalized on host)
    out = nc.dram_tensor(
        "out", [HEADS_PER_CORE, D1, S], f32, kind="ExternalOutput"
    ).ap()


    with tile.TileContext(nc) as tc, ExitStack() as ctx:
        const = ctx.enter_context(tc.tile_pool(name="const", bufs=1))
        persist = ctx.enter_context(tc.tile_pool(name="persist", bufs=1))
        xpool = ctx.enter_context(tc.tile_pool(name="xpool", bufs=2))
        mpool = ctx.enter_context(tc.tile_pool(name="mpool", bufs=2))
        fpool = ctx.enter_context(tc.tile_pool(name="fpool", bufs=2))
        ppool = ctx.enter_context(tc.tile_pool(name="ppool", bufs=2))
        copool = ctx.enter_context(tc.tile_pool(name="copool", bufs=2))
        spool = ctx.enter_context(tc.tile_pool(name="spool", bufs=3))
        # PSUM: 5 shared (proj + scores) + 3 (ctx) = 8 banks
        psP = ctx.enter_context(tc.tile_pool(name="psP", bufs=5, space="PSUM"))
        psC = psP
        psD = ctx.enter_context(tc.tile_pool(name="psD", bufs=3, space="PSUM"))

        # --- constants (DMA in critical-path order: K-proj needs wk first,
        # then Q0, then mask chunk 0; V-path items come after) --------------
        wk_sb = const.tile([128, KC, F], bf16)
        nc.sync.dma_start(wk_sb[:], wkT)
        wq_sb = const.tile([128, KC, F], bf16)
        nc.sync.dma_start(wq_sb[:], wqT)
        bk_sb = const.tile([128, 2], f32)
        nc.sync.dma_start(bk_sb[:], bks)
        bq_sb = const.tile([128, 2], f32)
        nc.sync.dma_start(bq_sb[:], bqs)
        m0 = mpool.tile([128, NT, SQC], bf16, tag="m", name="mask_0")
        wv_sb = const.tile([128, KC, F], bf16)
        bv_sb = const.tile([1, F], f32)

        ones1 = const.tile([1, 128], f32)
        nc.vector.memset(ones1[:], 1.0)
        bvRep = const.tile([128, F], f32)

        QT_sb = persist.tile([128, 2, S], bf16)
        # K.T zero-padded per head: head h lives in rows (h%2)*64..+64 of
        # KTz[:, h, :], the other 64 rows stay zero -> scores run at K=128.
        KTz = persist.tile([128, HEADS_PER_CORE, S], bf16)
        for h in range(HEADS_PER_CORE):
            zr = (1 - h % 2) * 64
            nc.gpsimd.memset(KTz[zr : zr + 64, h, :], 0.0)
        V_sb = persist.tile([128, NT, HEADS_PER_CORE * D1], bf16)
        V4 = V_sb.rearrange("p t (h j) -> p t h j", h=HEADS_PER_CORE)
        nc.gpsimd.memset(V4[:, :, :, D_HEAD : D_HEAD + 1], 1.0)

        # --- K projection (+ head-split into KTz halves, bias on ACT) ------
        for sc in range(NSC):
            xk_t = xpool.tile([128, KC, SQC], bf16, tag="xk", bufs=3)
            nc.sync.dma_start(xk_t[:], xkT[sc])
            for ft in range(2):
                ps = psP.tile([128, SQC], f32, tag="pp")
                for kc in range(KC):
                    nc.tensor.matmul(
                        ps[:],
                        wk_sb[:, kc, ft * 128 : (ft + 1) * 128],
                        xk_t[:, kc, :],
                        start=(kc == 0),
                        stop=(kc == KC - 1),
                    )
                sl = slice(sc * SQC, (sc + 1) * SQC)
                nc.scalar.activation(
                    KTz[0:64, 2 * ft, sl], ps[0:64, :], Ident, bias=bk_sb[0:64, ft : ft + 1]
                )
                nc.scalar.activation(
                    KTz[64:128, 2 * ft + 1, sl],
                    ps[64:128, :],
                    Ident,
                    bias=bk_sb[64:128, ft : ft + 1],
                )

        # --- Q projection chunk maker (bias+scale on ACT) ------------------
        def q_load(sc):
            xq_t = xpool.tile([128, KC, SQC], bf16, tag="xt", name=f"xq_{sc}")
            nc.sync.dma_start(xq_t[:], xqT[sc])
            return xq_t

        def q_proj(sc, xq_t=None):
            if xq_t is None:
                xq_t = q_load(sc)
            for ft in range(2):
                ps = psP.tile([128, SQC], f32, tag="pp", name=f"psq_{sc}_{ft}")
                for kc in range(KC):
                    nc.tensor.matmul(
                        ps[:],
                        wq_sb[:, kc, ft * 128 : (ft + 1) * 128],
                        xq_t[:, kc, :],
                        start=(kc == 0),
                        stop=(kc == KC - 1),
                    )
                # out = in*0.125 + bq (bq pre-scaled by 0.125 on host)
                nc.scalar.activation(
                    QT_sb[:, ft, sc * SQC : (sc + 1) * SQC],
                    ps[:],
                    Ident,
                    bias=bq_sb[:, ft : ft + 1],
                    scale=0.125,
                )

        def v_proj(sc):
            # V projection (bias add on DVE via replicated-bias trick)
            xv_t = xpool.tile([128, KC, SQC], bf16, tag="xt", name=f"xv_{sc}")
            nc.sync.dma_start(xv_t[:], xvT[sc])
            for st in range(SQC // 128):
                t = sc * (SQC // 128) + st
                ps = psP.tile([128, F], f32, tag="pp", name=f"psv_{sc}_{st}")
                for kc in range(KC):
                    nc.tensor.matmul(
                        ps[:],
                        xv_t[:, kc, st * 128 : (st + 1) * 128],
                        wv_sb[:, kc, :],
                        start=(kc == 0),
                        stop=(kc == KC - 1),
                    )
                nc.vector.tensor_add(
                    V4[:, t, :, 0:D_HEAD],
                    ps[:].rearrange("p (h j) -> p h j", h=HEADS_PER_CORE),
                    bvRep.rearrange("p (h j) -> p h j", h=HEADS_PER_CORE),
                )

        def attn_scores(sc, h, m_t, eg=EG):
            # scores.T * mask -> exp : probs tiles [128 sk, NT, SQC] bf16.
            # A few tiles per head detour via ACT-copy + GpSimd multiply to
            # offload the DVE (the overall bottleneck).
            ft = h // 2
            gp_tiles = {5}
            probs = ppool.tile([128, NT, SQC], bf16, tag="probs", name=f"probs_{sc}_{h}")
            for g in range((NT + eg - 1) // eg):
                t0 = g * eg
                t1 = min(NT, t0 + eg)
                pf = fpool.tile([128, eg, SQC], f32, tag="pf", name=f"pf_{sc}_{h}_{g}")
                for t in range(t0, t1):
                    ps = psC.tile([128, SQC], f32, tag="pp", name=f"pss_{sc}_{h}_{t}")
                    nc.tensor.matmul(
                        ps[:],
                        KTz[:, h, t * 128 : (t + 1) * 128],
                        QT_sb[:, ft, sc * SQC : (sc + 1) * SQC],
                        start=True,
                        stop=True,
                    )
                    if t in gp_tiles:
                        ss = spool.tile([128, SQC], bf16, tag="ss", name=f"ss_{sc}_{h}_{t}")
                        nc.scalar.copy(ss[:], ps[:])
                        nc.gpsimd.tensor_mul(pf[:, t - t0, :], ss[:], m_t[:, t, :])
                    else:
                        nc.vector.tensor_mul(pf[:, t - t0, :], ps[:], m_t[:, t, :])
                nc.scalar.activation(probs[:, t0:t1, :], pf[:, 0 : t1 - t0, :], Exp)
            return probs

        def attn_av(sc, h, probs):
            # ctx.T = (V|1).T @ probs.T : V|1 stationary, probs moving
            pc = psD.tile([D1, SQC], f32, tag="psc", name=f"psc_{sc}_{h}")
            for t in range(NT):
                nc.tensor.matmul(
                    pc[:],
                    V_sb[:, t, h * D1 : (h + 1) * D1],
                    probs[:, t, :],
                    start=(t == 0),
                    stop=(t == NT - 1),
                )
            co = copool.tile([D1, SQC], f32, tag="co", name=f"co_{sc}_{h}")
            nc.scalar.copy(co[:], pc[:])
            nc.sync.dma_start(out[h, :, sc * SQC : (sc + 1) * SQC], co[:])

        # Head 0 of chunk 0 starts as soon as K-proj + Q(0) + mask(0) are
        # ready; V projection overlaps its mask-multiplies on the DVE, and
        # each AV matmul is software-pipelined one head behind its scores so
        # the PE never stalls the DVE at head/chunk boundaries.
        q_proj(0)
        nc.sync.dma_start(m0[:], maskT[0])
        nc.sync.dma_start(wv_sb[:], wvT)
        nc.sync.dma_start(bv_sb[:], bvr)
        bvp = psP.tile([128, F], f32, tag="pp")
        nc.tensor.matmul(bvp[:], ones1[:], bv_sb[:], start=True, stop=True)
        nc.vector.tensor_copy(bvRep[:], bvp[:])
        probs00 = attn_scores(0, 0, m0)
        for sc in range(NSC):
            v_proj(sc)
        pend = (0, 0, probs00)
        masks = {0: m0}
        last_hd = (NSC - 1, HEADS_PER_CORE - 1)
        for sc in range(NSC):
            m_t = masks[sc]
            for h in range(HEADS_PER_CORE):
                if sc == 0 and h == 0:
                    continue
                if h == 1 and sc + 1 < NSC:
                    # prefetch next chunk's mask while this chunk computes
                    mn = mpool.tile([128, NT, SQC], bf16, tag="m", name=f"mask_{sc+1}")
                    nc.sync.dma_start(mn[:], maskT[sc + 1])
                    masks[sc + 1] = mn
                if h == 2 and sc + 1 < NSC:
                    # project next chunk's Q mid-chunk so the PE isn't doing
                    # it right when the next chunk's scores are needed
                    q_proj(sc + 1)
                # finer exp batches on the last head shorten the drain tail
                p = attn_scores(sc, h, m_t, eg=(4 if (sc, h) == last_hd else EG))
                attn_av(*pend)
                pend = (sc, h, p)
        attn_av(*pend)

    nc.compile()
    return nc


def _x_slab(xb, S):
    """[S, D] input -> [NSC, 128, KC, SQC] bf16 per-chunk contiguous slab
    holding x.T tiles: slab[sc, p, c, j] = x[sc*SQC+j, c*128+p]."""
    SQC = min(512, S)
    NSC = S // SQC
    # x.T [D, S] -> (c p) (sc j)
    xt = xb.T.reshape(KC, 128, NSC, SQC)
    return np.ascontiguousarray(xt.transpose(2, 1, 0, 3)).astype(BF16)


def _w_slab(w):
    """[F, D] weight slice -> [128, KC, F] bf16: slab[p, c, f] = w[f, c*128+p]."""
    wt = w.T.reshape(KC, 128, F)
    return np.ascontiguousarray(wt.transpose(1, 0, 2)).astype(BF16)


def _mask_slab(mb, S):
    """[S, S] mask -> [NSC, 128, NT, SQC] f32 slab of mask.T tiles:
    slab[sc, p, t, j] = mask.T[t*128+p, sc*SQC+j] = mask[sc*SQC+j, t*128+p]."""
    SQC = min(512, S)
    NSC = S // SQC
    NT = S // 128
    mt = mb.T.reshape(NT, 128, NSC, SQC)
    return np.ascontiguousarray(mt.transpose(2, 1, 0, 3)).astype(BF16)


def make_in_maps(q, k, v, mask, Wq, bq, Wk, bk, Wv, bv):
    """Build the 8 per-core input maps (host-side shard + transpose + cast)."""
    S = q.shape[1]
    xT = {}
    mT = {}
    for b in range(B):
        xT[("q", b)] = _x_slab(q[b], S)
        xT[("k", b)] = _x_slab(k[b], S)
        xT[("v", b)] = _x_slab(v[b], S)
        mT[b] = _mask_slab(mask[b, 0], S)

    in_maps = []
    for c in range(N_CORES):
        b = c // 4
        g = c % 4
        fs = slice(g * F, (g + 1) * F)
        in_maps.append(
            {
                "xqT": xT[("q", b)],
                "xkT": xT[("k", b)],
                "xvT": xT[("v", b)],
                "wqT": _w_slab(Wq[fs]),
                "wkT": _w_slab(Wk[fs]),
                "wvT": _w_slab(Wv[fs]),
                "bqs": np.ascontiguousarray(
                    (bq[fs] * 0.125).reshape(2, 128).T
                ).astype(np.float32),
                "bks": np.ascontiguousarray(bk[fs].reshape(2, 128).T).astype(
                    np.float32
                ),
                "bvr": np.ascontiguousarray(bv[fs].reshape(1, F)).astype(np.float32),
                "maskT": mT[b],
            }
        )
    return in_maps


def assemble_output(results, S=2048):
    ctx_all = np.empty((B, N_HEAD, S, D_HEAD), np.float32)
    for c in range(N_CORES):
        b = c // 4
        g = c % 4
        oc = results[c]["out"]  # [4, 65, S]: rows 0..63 ctx.T, row 64 denom
        for hl in range(HEADS_PER_CORE):
            ctx_all[b, g * 4 + hl] = (oc[hl, :D_HEAD, :] / oc[hl, D_HEAD:, :]).T
    return ctx_all.reshape(B, S, N_HEAD * D_HEAD)


def run_cores(in_maps, trace=False, **kwargs):
    if "nc" not in _NC_CACHE:
        _NC_CACHE["nc"] = build_nc()
    nc = _NC_CACHE["nc"]
    return run_bass_kernel_spmd(
        nc, in_maps, core_ids=list(range(N_CORES)), trace=trace, **kwargs
    )


def kernel(q, k, v, mask, Wq, bq, Wk, bk, Wv, bv):
    q = np.asarray(q, dtype=np.float32)
    k = np.asarray(k, dtype=np.float32)
    v = np.asarray(v, dtype=np.float32)
    mask = np.asarray(mask, dtype=np.float32)
    in_maps = make_in_maps(
        q,
        k,
        v,
        mask,
        np.asarray(Wq, np.float32),
        np.asarray(bq, np.float32),
        np.asarray(Wk, np.float32),
        np.asarray(bk, np.float32),
        np.asarray(Wv, np.float32),
        np.asarray(bv, np.float32),
    )
    res = run_cores(in_maps)
    return assemble_output(res.results, S=q.shape[1])
